# revision 3
# baseline (speedup 1.0000x reference)
"""CAM (channel attention) kernel v2 for Trainium2, 8-core data-parallel.

Per batch item (one per NeuronCore):
    energy   = Q @ K^T                     (C x C, contract over N)
    att      = softmax(max(energy) - energy) = softmax(-energy)
    out      = gamma * (att @ V) + V

v2 design vs baseline: all q/k transposes moved off the DMA xbar onto the
TensorEngine (identity matmuls, bf16), so the DMA queues carry ONLY the
32MB of compulsory HBM traffic (24MB loads + 8MB stores).

  - q,k streamed in [128, GW] f32 chunks; q on the SP HWDGE ring, k on the
    ACT ring. casts f32->bf16 on DVE (q) / ACT (k).
  - per n-chunk j: 8 identity matmuls transpose the four 128-col blocks of
    q and k into two PSUM staging tiles [128, 4*128] bf16; DVE/ACT copy
    them to SBUF (qTs/kTs), then 4 accumulating bf16 matmuls add the
    j-chunk's contribution to energy[c] (4 PSUM banks, [128c, 512d]).
  - v loads ([128,4096] f32) are queued on the SP ring AFTER the q chunks
    so they can't starve the energy phase's loads.
  - softmax over the free dim: DVE min, ACT exp(bias=rowmin, scale=-1)
    with fused row-sum, DVE reciprocal; gamma folded into the scale.
    att stays f32.
  - attT via f32 identity matmuls (16 blocks), then matmul2 in f32r:
    lhsT = attT slices, rhs = v f32 slices bitcast to f32r (1 cyc/row at
    N=512) -- v is never cast or copied. epilogue out = psum + v on DVE,
    stores [128, 4, 512] (1MB) on the ACT ring.
"""

import numpy as np

B, C, H, W = 8, 512, 64, 64
N = H * W  # 4096
P = 128
CT = C // P  # 4 c-tiles
NJ = N // P  # 32 n-chunks
NO = N // 512  # 8 output column chunks

_nc_cache: dict = {}


def _body(nc, tc, cfg):
    from contextlib import ExitStack

    import concourse.mybir as mybir
    from concourse.bass import ts
    from concourse.masks import make_identity

    cfg = cfg or {}
    do = lambda phase: phase not in cfg.get("skip", ())

    dt = mybir.dt
    f32, bf16 = dt.float32, dt.bfloat16
    X = mybir.AxisListType.X

    NG = cfg.get("ng", 4)  # n-groups for q,k loads
    GJ = NJ // NG  # n-chunks per group
    GW = GJ * P  # columns per load chunk

    qa = nc.kio["q"].ap().rearrange("(a p) w -> a p w", p=P)
    ka = nc.kio["k"].ap().rearrange("(a p) w -> a p w", p=P)
    va = nc.kio["v"].ap().rearrange("(a p) w -> a p w", p=P)
    ga = nc.kio["gamma"].ap()
    oa_p = nc.kio["out"].ap().rearrange("(a p) w -> p a w", p=P)

    with ExitStack() as ctx:
        ep = ctx.enter_context

        p_nat = ep(tc.tile_pool(name="nat", bufs=3))
        p_cast = ep(tc.tile_pool(name="cast", bufs=5))
        p_qkT = ep(tc.tile_pool(name="qkT", bufs=10))
        p_vf = ep(tc.tile_pool(name="vf", bufs=CT))
        p_att = ep(tc.tile_pool(name="att", bufs=CT))
        p_attT = ep(tc.tile_pool(name="attT", bufs=CT))
        p_small = ep(tc.tile_pool(name="small", bufs=4))
        p_misc = ep(tc.tile_pool(name="misc", bufs=1))
        p_es = ep(tc.tile_pool(name="es", bufs=3))
        p_vb = ep(tc.tile_pool(name="vb", bufs=CT))

        # gamma broadcast across partitions: [1,1] DRAM -> [128,1] SBUF
        g128 = p_misc.tile([P, 1], f32)
        nc.sync.dma_start(g128[:], ga.broadcast_to([P, 1]))

        ident_b = p_misc.tile([P, P], bf16)
        make_identity(nc, ident_b[:])

        att = []
        v_f = []
        with tc.tile_pool(name="energy", bufs=CT, space="PSUM") as p_energy, \
             tc.tile_pool(name="tst", bufs=4, space="PSUM") as p_tst:
            e_ps = [
                p_energy.tile([P, 512], f32, tag="e", name=f"e{c}")
                for c in range(CT)
            ]

            if do("loads_qk"):
                for g in range(NG):
                    qn_g, kn_g = [], []
                    for c in range(CT):
                        qn32 = p_nat.tile(
                            [P, GW], f32, tag="qn32", name=f"qn32_{g}{c}"
                        )
                        nc.sync.dma_start(qn32[:], qa[c][:, ts(g, GW)])
                        qn = p_cast.tile(
                            [P, GW], bf16, tag="qn", name=f"qn{g}{c}"
                        )
                        nc.vector.tensor_copy(qn[:], qn32[:])
                        qn_g.append(qn)
                        kn32 = p_nat.tile(
                            [P, GW], f32, tag="kn32", name=f"kn32_{g}{c}"
                        )
                        nc.sync.dma_start(kn32[:], ka[c][:, ts(g, GW)])
                        kn = p_cast.tile(
                            [P, GW], bf16, tag="kn", name=f"kn{g}{c}"
                        )
                        nc.scalar.copy(kn[:], kn32[:])
                        kn_g.append(kn)
                    if not (do("tpose") and do("mm1")):
                        continue
                    qTs_g, kTs_g = [], []
                    for jj in range(GJ):
                        qst = p_tst.tile([P, 512], f32, tag="tst")
                        kst = p_tst.tile([P, 512], f32, tag="tst")
                        for c in range(CT):
                            nc.tensor.matmul(
                                qst[:, ts(c, P)],
                                qn_g[c][:, ts(jj, P)],
                                ident_b[:],
                                start=True,
                                stop=True,
                            )
                            nc.tensor.matmul(
                                kst[:, ts(c, P)],
                                kn_g[c][:, ts(jj, P)],
                                ident_b[:],
                                start=True,
                                stop=True,
                            )
                        qTs = p_qkT.tile([P, 512], bf16, tag="qTs")
                        kTs = p_qkT.tile([P, 512], bf16, tag="kTs")
                        if jj % 2 == 0:
                            nc.vector.tensor_copy(qTs[:], qst[:])
                            nc.scalar.copy(kTs[:], kst[:])
                        else:
                            nc.scalar.copy(qTs[:], qst[:])
                            nc.vector.tensor_copy(kTs[:], kst[:])
                        qTs_g.append(qTs)
                        kTs_g.append(kTs)
                    for jj in range(GJ):
                        j = g * GJ + jj
                        for c in range(CT):
                            nc.tensor.matmul(
                                e_ps[c][:],
                                qTs_g[jj][:, ts(c, P)],
                                kTs_g[jj][:],
                                start=(j == 0),
                                stop=(j == NJ - 1),
                            )
                    # v loads ride the same SP ring; vg picks after which
                    # q,k group they queue (NG-1 = after all q,k)
                    if do("loads_v") and g == min(cfg.get("vg", NG - 1), NG - 1):
                        for d in range(CT):
                            vf = p_vf.tile(
                                [P, N], f32, tag="vf", name=f"vf{d}"
                            )
                            nc.sync.dma_start(vf[:], va[d])
                            v_f.append(vf)

            if not (do("loads_qk") and do("tpose") and do("mm1")):
                return

            # softmax(-energy) rows, gamma folded into the normalization
            for c in range(CT):
                rowmin = p_small.tile([P, 1], f32)
                nc.vector.tensor_reduce(
                    rowmin[:], e_ps[c][:], axis=X, op=mybir.AluOpType.min
                )
                att_c = p_att.tile([P, 512], bf16, tag="att", name=f"att{c}")
                rowsum = p_small.tile([P, 1], f32)
                nc.scalar.activation(
                    att_c[:],
                    e_ps[c][:],
                    mybir.ActivationFunctionType.Exp,
                    bias=rowmin[:, 0:1],
                    scale=-1.0,
                    accum_out=rowsum[:, 0:1],
                )
                recip = p_small.tile([P, 1], f32)
                nc.vector.reciprocal(recip[:], rowsum[:])
                srow = p_small.tile([P, 1], f32)
                nc.vector.tensor_scalar_mul(srow[:], recip[:], g128[:, 0:1])
                nc.vector.tensor_scalar_mul(att_c[:], att_c[:], srow[:, 0:1])
                att.append(att_c)

        if not do("mm2"):
            return

        # transpose att via PE identity matmuls into attT[d][:, c-block]
        attT = []
        with tc.tile_pool(name="pst", bufs=2, space="PSUM") as p_pst:
            for d in range(CT):
                pst = p_pst.tile([P, 512], f32, tag="pst")
                for c in range(CT):
                    nc.tensor.matmul(
                        pst[:, ts(c, P)],
                        att[c][:, ts(d, P)],
                        ident_b[:],
                        start=True,
                        stop=True,
                    )
                at = p_attT.tile([P, 512], bf16, tag="attT", name=f"attT{d}")
                if d % 2 == 0:
                    nc.vector.tensor_copy(at[:], pst[:])
                else:
                    nc.scalar.copy(at[:], pst[:])
                attT.append(at)

        # v -> bf16, split per c-tile across DVE/ACT (runs as each v lands)
        vb = []
        for d in range(CT):
            vbt = p_vb.tile([P, N], bf16, tag="vb", name=f"vb{d}")
            nc.vector.tensor_copy(vbt[:, 0 : N // 2], v_f[d][:, 0 : N // 2])
            nc.scalar.copy(vbt[:, N // 2 : N], v_f[d][:, N // 2 : N])
            vb.append(vbt)

        mm2_loop = cfg.get("mm2_loop", "no8")  # no8 | dpair
        with tc.tile_pool(name="ps2", bufs=8, space="PSUM") as p_ps2:
            if mm2_loop == "no8":
                # dense no-major: 8 banks double-buffer across output chunks
                for no in range(NO):
                    es = p_es.tile([P, CT, 512], f32)
                    for c in range(CT):
                        ps2 = p_ps2.tile(
                            [P, 512], f32, tag="ps2", name=f"ps2_{no}_{c}"
                        )
                        for d in range(CT):
                            nc.tensor.matmul(
                                ps2[:],
                                attT[d][:, ts(c, P)],
                                vb[d][:, ts(no, 512)],
                                start=(d == 0),
                                stop=(d == CT - 1),
                            )
                        nc.vector.tensor_add(
                            es[:, c, :], ps2[:], v_f[c][:, ts(no, 512)]
                        )
                    nc.scalar.dma_start(oa_p[:, :, ts(no, 512)], es[:])
            else:
                # d-major over pairs of output chunks: all 8 PSUM banks hold
                # the pair's (2 no) x (4 c) accumulators, so d<3 matmuls run
                # while later v tiles are still loading.
                for pr in range(NO // 2):
                    ps = [
                        [
                            p_ps2.tile(
                                [P, 512],
                                f32,
                                tag="ps2",
                                name=f"ps2_{pr}_{t}_{c}",
                            )
                            for c in range(CT)
                        ]
                        for t in range(2)
                    ]
                    for d in range(CT):
                        for t in range(2):
                            no = 2 * pr + t
                            for c in range(CT):
                                nc.tensor.matmul(
                                    ps[t][c][:],
                                    attT[d][:, ts(c, P)],
                                    vb[d][:, ts(no, 512)],
                                    start=(d == 0),
                                    stop=(d == CT - 1),
                                )
                    for t in range(2):
                        no = 2 * pr + t
                        es = p_es.tile([P, CT, 512], f32)
                        for c in range(CT):
                            nc.vector.tensor_add(
                                es[:, c, :], ps[t][c][:], v_f[c][:, ts(no, 512)]
                            )
                        nc.scalar.dma_start(oa_p[:, :, ts(no, 512)], es[:])


def build(repeat=1, cfg=None, loop_n=None):
    import concourse.mybir as mybir
    import concourse.tile as tile
    from concourse import bacc

    dt = mybir.dt
    nc = bacc.Bacc("TRN2", target_bir_lowering=False, debug=False)
    nc.kio = {}
    for name in ("q", "k", "v"):
        nc.kio[name] = nc.dram_tensor(
            name, [C, N], dt.float32, kind="ExternalInput"
        )
    nc.kio["gamma"] = nc.dram_tensor(
        "gamma", [1, 1], dt.float32, kind="ExternalInput"
    )
    nc.kio["out"] = nc.dram_tensor(
        "out", [C, N], dt.float32, kind="ExternalOutput"
    )
    with tile.TileContext(nc) as tc:
        if loop_n is not None:
            with tc.For_i(0, loop_n, 1):
                _body(nc, tc, cfg)
        else:
            for _ in range(repeat):
                _body(nc, tc, cfg)
    nc.compile()
    return nc


def _get_nc():
    if "nc" not in _nc_cache:
        _nc_cache["nc"] = build(repeat=1)
    return _nc_cache["nc"]


def make_in_maps(q, k, v, gamma):
    q = np.ascontiguousarray(np.asarray(q, dtype=np.float32).reshape(B, C, N))
    k = np.ascontiguousarray(np.asarray(k, dtype=np.float32).reshape(B, C, N))
    v = np.ascontiguousarray(np.asarray(v, dtype=np.float32).reshape(B, C, N))
    g = np.asarray(gamma, dtype=np.float32).reshape(1, 1)
    return [
        {"q": q[i], "k": k[i], "v": v[i], "gamma": g} for i in range(B)
    ]


def kernel(q, k, v, gamma):
    from concourse import bass_utils

    nc = _get_nc()
    in_maps = make_in_maps(q, k, v, gamma)
    res = bass_utils.run_bass_kernel_spmd(nc, in_maps, core_ids=list(range(B)))
    out = np.stack([res.results[i]["out"] for i in range(B)])
    return out.reshape(B, C, H, W).astype(np.float32, copy=False)


# revision 4
# speedup vs baseline: 1.0877x; 1.0877x over previous
"""CAM (channel attention) kernel v2 for Trainium2, 8-core data-parallel.

Per batch item (one per NeuronCore):
    energy   = Q @ K^T                     (C x C, contract over N)
    att      = softmax(max(energy) - energy) = softmax(-energy)
    out      = gamma * (att @ V) + V

Design vs the xbar baseline: all q/k transposes are done on the
TensorEngine (identity matmuls, bf16), so the DMA queues carry ONLY the
32MB of compulsory HBM traffic (24MB loads + 8MB stores). ~128us/iter vs
the 293us xbar baseline (loads ~83us at ~290GB/s + mm2/store tail).

  - ALL loads ride the SP HWDGE ring (SP runs no compute, so DMA issues
    are never stuck behind compute in an engine FIFO; ring FIFO order =
    q,k group-interleaved first, v last). casts f32->bf16 on DVE (q) /
    ACT (k).
  - per group g of 8 n-chunks: for each chunk j, 8 identity matmuls
    transpose the four 128-col blocks of q and k into two PSUM staging
    tiles [128, 4*128] f32; DVE/ACT drain them to bf16 SBUF (qTs/kTs);
    then the group's 32 accumulating bf16 matmuls add its contribution
    to energy[c] (4 PSUM banks, [128c, 512d]). Group-blocked emission
    (all transposes, then all mm1) keeps every engine FIFO in one global
    order -- per-chunk interleave deadlocks the Tile scheduler.
  - softmax over the free dim: DVE min, ACT exp(bias=rowmin, scale=-1)
    with fused row-sum, DVE reciprocal; gamma folded into the scale;
    att in bf16.
  - attT via PE identity matmuls; v pre-cast to bf16 (split DVE/ACT) as
    each v tile lands, off matmul2's critical path.
  - matmul2 no-major with all 8 PSUM banks (dense PE work, no HAM
    cooldown): 16 bf16 matmuls per 512-col output chunk; epilogue
    out = psum + v (f32, exact for gamma=0) on DVE; [128, 4, 512] 1MB
    stores on the ACT ring.
"""

import numpy as np

B, C, H, W = 8, 512, 64, 64
N = H * W  # 4096
P = 128
CT = C // P  # 4 c-tiles
NJ = N // P  # 32 n-chunks
NO = N // 512  # 8 output column chunks

_nc_cache: dict = {}


def _body(nc, tc, cfg):
    from contextlib import ExitStack

    import concourse.mybir as mybir
    from concourse.bass import ts
    from concourse.masks import make_identity

    cfg = cfg or {}
    do = lambda phase: phase not in cfg.get("skip", ())

    dt = mybir.dt
    f32, bf16 = dt.float32, dt.bfloat16
    X = mybir.AxisListType.X

    NG = cfg.get("ng", 4)  # n-groups for q,k loads
    GJ = NJ // NG  # n-chunks per group
    GW = GJ * P  # columns per load chunk

    qa = nc.kio["q"].ap().rearrange("(a p) w -> a p w", p=P)
    ka = nc.kio["k"].ap().rearrange("(a p) w -> a p w", p=P)
    va = nc.kio["v"].ap().rearrange("(a p) w -> a p w", p=P)
    ga = nc.kio["gamma"].ap()
    oa_p = nc.kio["out"].ap().rearrange("(a p) w -> p a w", p=P)

    with ExitStack() as ctx:
        ep = ctx.enter_context

        big = NG <= 2  # wide chunks: shrink pools to fit SBUF
        p_nat = ep(tc.tile_pool(name="nat", bufs=2 if big else 3))
        p_cast = ep(tc.tile_pool(name="cast", bufs=4 if big else 5))
        p_qkT = ep(tc.tile_pool(name="qkT", bufs=8 if big else 10))
        p_vf = ep(tc.tile_pool(name="vf", bufs=CT))
        p_att = ep(tc.tile_pool(name="att", bufs=CT))
        p_attT = ep(tc.tile_pool(name="attT", bufs=CT))
        p_small = ep(tc.tile_pool(name="small", bufs=4))
        p_misc = ep(tc.tile_pool(name="misc", bufs=1))
        p_es = ep(tc.tile_pool(name="es", bufs=2 if NG <= 2 else 3))
        p_vb = ep(tc.tile_pool(name="vb", bufs=CT))

        # gamma broadcast across partitions: [1,1] DRAM -> [128,1] SBUF
        g128 = p_misc.tile([P, 1], f32)
        nc.sync.dma_start(g128[:], ga.broadcast_to([P, 1]))

        ident_b = p_misc.tile([P, P], bf16)
        make_identity(nc, ident_b[:])

        att = []
        v_f = []
        with tc.tile_pool(name="energy", bufs=CT, space="PSUM") as p_energy, \
             tc.tile_pool(name="tst", bufs=4, space="PSUM") as p_tst:
            e_ps = [
                p_energy.tile([P, 512], f32, tag="e", name=f"e{c}")
                for c in range(CT)
            ]

            if do("loads_qk"):
                for g in range(NG):
                    qn_g, kn_g = [], []
                    for c in range(CT):
                        qn32 = p_nat.tile(
                            [P, GW], f32, tag="qn32", name=f"qn32_{g}{c}"
                        )
                        nc.sync.dma_start(qn32[:], qa[c][:, ts(g, GW)])
                        qn = p_cast.tile(
                            [P, GW], bf16, tag="qn", name=f"qn{g}{c}"
                        )
                        nc.vector.tensor_copy(qn[:], qn32[:])
                        qn_g.append(qn)
                        kn32 = p_nat.tile(
                            [P, GW], f32, tag="kn32", name=f"kn32_{g}{c}"
                        )
                        nc.sync.dma_start(kn32[:], ka[c][:, ts(g, GW)])
                        kn = p_cast.tile(
                            [P, GW], bf16, tag="kn", name=f"kn{g}{c}"
                        )
                        nc.scalar.copy(kn[:], kn32[:])
                        kn_g.append(kn)
                    if not (do("tpose") and do("mm1")):
                        continue
                    qTs_g, kTs_g = [], []
                    for jj in range(GJ):
                        qst = p_tst.tile([P, 512], f32, tag="tst")
                        kst = p_tst.tile([P, 512], f32, tag="tst")
                        for c in range(CT):
                            nc.tensor.matmul(
                                qst[:, ts(c, P)],
                                qn_g[c][:, ts(jj, P)],
                                ident_b[:],
                                start=True,
                                stop=True,
                            )
                            nc.tensor.matmul(
                                kst[:, ts(c, P)],
                                kn_g[c][:, ts(jj, P)],
                                ident_b[:],
                                start=True,
                                stop=True,
                            )
                        qTs = p_qkT.tile([P, 512], bf16, tag="qTs")
                        kTs = p_qkT.tile([P, 512], bf16, tag="kTs")
                        if jj % 2 == 0:
                            nc.vector.tensor_copy(qTs[:], qst[:])
                            nc.scalar.copy(kTs[:], kst[:])
                        else:
                            nc.scalar.copy(qTs[:], qst[:])
                            nc.vector.tensor_copy(kTs[:], kst[:])
                        qTs_g.append(qTs)
                        kTs_g.append(kTs)
                    for jj in range(GJ):
                        j = g * GJ + jj
                        for c in range(CT):
                            nc.tensor.matmul(
                                e_ps[c][:],
                                qTs_g[jj][:, ts(c, P)],
                                kTs_g[jj][:],
                                start=(j == 0),
                                stop=(j == NJ - 1),
                            )
                    # v loads ride the same SP ring; vg picks after which
                    # q,k group they queue (NG-1 = after all q,k)
                    if do("loads_v") and g == min(cfg.get("vg", NG - 1), NG - 1):
                        for d in range(CT):
                            vf = p_vf.tile(
                                [P, N], f32, tag="vf", name=f"vf{d}"
                            )
                            nc.sync.dma_start(vf[:], va[d])
                            v_f.append(vf)

            if not (do("loads_qk") and do("tpose") and do("mm1")):
                return

            # softmax(-energy) rows, gamma folded into the normalization
            for c in range(CT):
                rowmin = p_small.tile([P, 1], f32)
                nc.vector.tensor_reduce(
                    rowmin[:], e_ps[c][:], axis=X, op=mybir.AluOpType.min
                )
                att_c = p_att.tile([P, 512], bf16, tag="att", name=f"att{c}")
                rowsum = p_small.tile([P, 1], f32)
                nc.scalar.activation(
                    att_c[:],
                    e_ps[c][:],
                    mybir.ActivationFunctionType.Exp,
                    bias=rowmin[:, 0:1],
                    scale=-1.0,
                    accum_out=rowsum[:, 0:1],
                )
                recip = p_small.tile([P, 1], f32)
                nc.vector.reciprocal(recip[:], rowsum[:])
                srow = p_small.tile([P, 1], f32)
                nc.vector.tensor_scalar_mul(srow[:], recip[:], g128[:, 0:1])
                nc.vector.tensor_scalar_mul(att_c[:], att_c[:], srow[:, 0:1])
                att.append(att_c)

        if not do("mm2"):
            return

        # transpose att via PE identity matmuls into attT[d][:, c-block]
        attT = []
        with tc.tile_pool(name="pst", bufs=2, space="PSUM") as p_pst:
            for d in range(CT):
                pst = p_pst.tile([P, 512], f32, tag="pst")
                for c in range(CT):
                    nc.tensor.matmul(
                        pst[:, ts(c, P)],
                        att[c][:, ts(d, P)],
                        ident_b[:],
                        start=True,
                        stop=True,
                    )
                at = p_attT.tile([P, 512], bf16, tag="attT", name=f"attT{d}")
                if d % 2 == 0:
                    nc.vector.tensor_copy(at[:], pst[:])
                else:
                    nc.scalar.copy(at[:], pst[:])
                attT.append(at)

        # v -> bf16, split per c-tile across DVE/ACT (runs as each v lands)
        vb = []
        for d in range(CT):
            vbt = p_vb.tile([P, N], bf16, tag="vb", name=f"vb{d}")
            nc.vector.tensor_copy(vbt[:, 0 : N // 2], v_f[d][:, 0 : N // 2])
            nc.scalar.copy(vbt[:, N // 2 : N], v_f[d][:, N // 2 : N])
            vb.append(vbt)

        mm2_loop = cfg.get("mm2_loop", "no8")  # no8 | dpair
        with tc.tile_pool(name="ps2", bufs=8, space="PSUM") as p_ps2:
            if mm2_loop == "no8":
                # dense no-major: 8 banks double-buffer across output chunks
                for no in range(NO):
                    es = p_es.tile([P, CT, 512], f32)
                    for c in range(CT):
                        ps2 = p_ps2.tile(
                            [P, 512], f32, tag="ps2", name=f"ps2_{no}_{c}"
                        )
                        for d in range(CT):
                            nc.tensor.matmul(
                                ps2[:],
                                attT[d][:, ts(c, P)],
                                vb[d][:, ts(no, 512)],
                                start=(d == 0),
                                stop=(d == CT - 1),
                            )
                        nc.vector.tensor_add(
                            es[:, c, :], ps2[:], v_f[c][:, ts(no, 512)]
                        )
                    nc.scalar.dma_start(oa_p[:, :, ts(no, 512)], es[:])
            else:
                # d-major over pairs of output chunks: all 8 PSUM banks hold
                # the pair's (2 no) x (4 c) accumulators, so d<3 matmuls run
                # while later v tiles are still loading.
                for pr in range(NO // 2):
                    ps = [
                        [
                            p_ps2.tile(
                                [P, 512],
                                f32,
                                tag="ps2",
                                name=f"ps2_{pr}_{t}_{c}",
                            )
                            for c in range(CT)
                        ]
                        for t in range(2)
                    ]
                    for d in range(CT):
                        for t in range(2):
                            no = 2 * pr + t
                            for c in range(CT):
                                nc.tensor.matmul(
                                    ps[t][c][:],
                                    attT[d][:, ts(c, P)],
                                    vb[d][:, ts(no, 512)],
                                    start=(d == 0),
                                    stop=(d == CT - 1),
                                )
                    for t in range(2):
                        no = 2 * pr + t
                        es = p_es.tile([P, CT, 512], f32)
                        for c in range(CT):
                            nc.vector.tensor_add(
                                es[:, c, :], ps[t][c][:], v_f[c][:, ts(no, 512)]
                            )
                        nc.scalar.dma_start(oa_p[:, :, ts(no, 512)], es[:])


def build(repeat=1, cfg=None, loop_n=None):
    import concourse.mybir as mybir
    import concourse.tile as tile
    from concourse import bacc

    dt = mybir.dt
    nc = bacc.Bacc("TRN2", target_bir_lowering=False, debug=False)
    nc.kio = {}
    for name in ("q", "k", "v"):
        nc.kio[name] = nc.dram_tensor(
            name, [C, N], dt.float32, kind="ExternalInput"
        )
    nc.kio["gamma"] = nc.dram_tensor(
        "gamma", [1, 1], dt.float32, kind="ExternalInput"
    )
    nc.kio["out"] = nc.dram_tensor(
        "out", [C, N], dt.float32, kind="ExternalOutput"
    )
    with tile.TileContext(nc) as tc:
        if loop_n is not None:
            with tc.For_i(0, loop_n, 1):
                _body(nc, tc, cfg)
        else:
            for _ in range(repeat):
                _body(nc, tc, cfg)
    nc.compile()
    return nc


def _get_nc():
    if "nc" not in _nc_cache:
        _nc_cache["nc"] = build(repeat=1)
    return _nc_cache["nc"]


def make_in_maps(q, k, v, gamma):
    q = np.ascontiguousarray(np.asarray(q, dtype=np.float32).reshape(B, C, N))
    k = np.ascontiguousarray(np.asarray(k, dtype=np.float32).reshape(B, C, N))
    v = np.ascontiguousarray(np.asarray(v, dtype=np.float32).reshape(B, C, N))
    g = np.asarray(gamma, dtype=np.float32).reshape(1, 1)
    return [
        {"q": q[i], "k": k[i], "v": v[i], "gamma": g} for i in range(B)
    ]


def kernel(q, k, v, gamma):
    from concourse import bass_utils

    nc = _get_nc()
    in_maps = make_in_maps(q, k, v, gamma)
    res = bass_utils.run_bass_kernel_spmd(nc, in_maps, core_ids=list(range(B)))
    out = np.stack([res.results[i]["out"] for i in range(B)])
    return out.reshape(B, C, H, W).astype(np.float32, copy=False)
